# revision 1
# baseline (speedup 1.0000x reference)
"""Trainium2 Bass kernel for nn_CSARoLayer (CNN + 2 RoPE-MHA layers + FFN).

Data-parallel over batch: 64 batches -> 8 NeuronCores x 8 batches.
Feature-major activation layout per batch: [H=4x128 partitions, S=512 free].
All matmuls run in float32r (fp22 truncated fp32, 1 cycle/row at N>=256).
"""

import ml_dtypes
import numpy as np

import concourse.bass as bass
import concourse.tile as tile
from concourse import mybir
from concourse.bass_utils import run_bass_kernel_spmd

# Problem constants (hardcoded per harness contract).
B, S, H, L, NH = 64, 512, 512, 2, 8
HD = H // NH
EPS = 1e-12
THETA = 10000.0
N_CORES = 8
B_LOCAL = B // N_CORES
P = 128
HC = H // P          # 4 hidden chunks
FC = 4 * H // P      # 16 ffn-hidden chunks
SCALE = 1.0 / float(np.sqrt(HD))

F32 = mybir.dt.float32
F32R = mybir.dt.float32r
AF = mybir.ActivationFunctionType
OP = mybir.AluOpType


def _split_waits(nc, max_waits=1):
    """This env's walrus rejects >2 sync commands per instruction. Hoist
    excess waits onto standalone EventSemaphore ops on the same engine,
    inserted immediately before the instruction (same-engine order is
    preserved, so semantics are unchanged)."""
    for f in nc.m.functions:
        for bb in f.blocks:
            newl = []
            for inst in bb.instructions:
                si = inst.sync_info
                if si is not None and si.on_wait and len(si.on_wait) > max_waits:
                    waits = list(si.on_wait)
                    keep = waits[-max_waits:]
                    extra = waits[:-max_waits]
                    for j, w in enumerate(extra):
                        nop = mybir.InstEventSemaphore(name=f"{inst.name}_sw{j}")
                        nop.engine = inst.engine
                        nop.sync_info = mybir.SyncInfo(on_wait=[w], on_update=[])
                        newl.append(nop)
                    inst.sync_info = mybir.SyncInfo(
                        on_wait=keep, on_update=list(si.on_update)
                    )
                newl.append(inst)
            bb.instructions = newl


def _layer_norm_feat(nc, pools, r_sb, ones_col, ones_row, out_sb, gb=None):
    """LayerNorm over features of feature-major r_sb [P, HC, S] -> out_sb.

    Stats per token via PE ones-matmuls; rstd = exp(-0.5*ln(var+eps));
    mean/rstd broadcast across partitions via K=1 matmuls; apply via
    full-tile tensor_tensor with stride-0 broadcast over the HC dim.
    gb: optional (g_sb, b_sb) tiles [P, HC] for non-trivial gain/bias.
    """
    big, sbuf, psum = pools
    r2_sb = big.tile([P, HC, S], F32R, name="ln_r2", tag="ln_r2")
    # per-chunk squares so the sumsq matmuls start after the first chunk
    # instead of after one monolithic full-tile op
    for hc in range(HC):
        nc.gpsimd.tensor_mul(r2_sb[:, hc, :], r_sb[:, hc, :], r_sb[:, hc, :])
    stats_a = psum.tile([1, S], F32, name="ln_stats_a", tag="pp_ln")
    stats_b = psum.tile([1, S], F32, name="ln_stats_b", tag="pp_ln")
    for hc in range(HC):
        nc.tensor.matmul(stats_a[0:1, :], ones_col[:, 0:1], r_sb[:, hc, :],
                         start=(hc == 0), stop=(hc == HC - 1))
    for hc in range(HC):
        nc.tensor.matmul(stats_b[0:1, :], ones_col[:, 0:1], r2_sb[:, hc, :],
                         start=(hc == 0), stop=(hc == HC - 1))
    m_sb = sbuf.tile([1, S], F32R, name="ln_m", tag="ln_m")
    ex2_sb = sbuf.tile([1, S], F32, name="ln_ex2", tag="ln_ex2")
    msq_sb = sbuf.tile([1, S], F32, name="ln_msq", tag="ln_msq")
    var_sb = sbuf.tile([1, S], F32, name="ln_var", tag="ln_var")
    lnv_sb = sbuf.tile([1, S], F32, name="ln_lnv", tag="ln_lnv")
    rstd_sb = sbuf.tile([1, S], F32R, name="ln_rstd", tag="ln_rstd")
    with nc.allow_low_precision(reason="f32r-tagged LN stats"):
        nc.vector.tensor_scalar_mul(m_sb, stats_a[0:1, :], 1.0 / H)
    nc.vector.tensor_scalar_mul(ex2_sb, stats_b[0:1, :], 1.0 / H)
    nc.vector.tensor_mul(msq_sb, m_sb, m_sb)
    nc.vector.tensor_sub(var_sb, ex2_sb, msq_sb)
    eps_sb = sbuf.tile([1, 1], F32, name="ln_eps", tag="ln_eps")
    nc.vector.memset(eps_sb, EPS)
    nc.scalar.activation(out=lnv_sb, in_=var_sb, func=AF.Ln,
                         bias=eps_sb[0:1, 0:1])
    with nc.allow_low_precision(reason="f32r-tagged LN rstd"):
        nc.scalar.activation(out=rstd_sb, in_=lnv_sb, func=AF.Exp, scale=-0.5)
    mb_ps = psum.tile([P, S], F32, name="ln_mb", tag="pp_ln")
    rb_ps = psum.tile([P, S], F32, name="ln_rb", tag="pp_ln")
    nc.tensor.matmul(mb_ps[:, :], ones_row[0:1, :], m_sb[0:1, :],
                     start=True, stop=True)
    nc.tensor.matmul(rb_ps[:, :], ones_row[0:1, :], rstd_sb[0:1, :],
                     start=True, stop=True)
    mb_sb = big.tile([P, S], F32, name="ln_mbs", tag="ln_mbs")
    rb_sb = big.tile([P, S], F32, name="ln_rbs", tag="ln_rbs")
    nc.scalar.activation(out=mb_sb, in_=mb_ps, func=AF.Copy)
    nc.scalar.activation(out=rb_sb, in_=rb_ps, func=AF.Copy)
    cen_sb = big.tile([P, HC, S], F32, name="ln_cen", tag="ln_r2")
    mb_b = mb_sb[:, None, :].to_broadcast((P, HC, S))
    rb_b = rb_sb[:, None, :].to_broadcast((P, HC, S))
    with nc.allow_low_precision(reason="f32r-tagged LN out"):
        if gb is None:
            # per-chunk apply: downstream matmuls consume chunk hc as soon
            # as it lands instead of waiting for the full tile
            for hc in range(HC):
                nc.gpsimd.tensor_sub(cen_sb[:, hc, :], r_sb[:, hc, :],
                                     mb_sb)
                nc.gpsimd.tensor_mul(out_sb[:, hc, :], cen_sb[:, hc, :],
                                     rb_sb)
        else:
            g_sb, b_sb = gb
            nc.gpsimd.tensor_sub(cen_sb, r_sb, mb_b)
            nc.gpsimd.tensor_mul(cen_sb, cen_sb, rb_b)
            for hc in range(HC):
                nc.vector.tensor_scalar(
                    out_sb[:, hc, :], cen_sb[:, hc, :],
                    g_sb[:, hc:hc + 1], b_sb[:, hc:hc + 1],
                    op0=OP.mult, op1=OP.add)


def build_nc(flags, split=True, gelu_identity=False):
    """Build the full SPMD program for one core (B_LOCAL batches)."""
    nc = bass.Bass()

    dram_in = {}

    def din(name, shape):
        dram_in[name] = nc.declare_dram_parameter(name, list(shape), F32,
                                                  isOutput=False)
        return dram_in[name]

    xT = din("xT", (B_LOCAL, P, HC, S))
    c1w = din("c1w", (P, HC, 5, P))        # conv1 lhsT taps (K=32 rows at 32r)
    c2w = nc.declare_dram_parameter("c2w", [P, HC, HC, 5, 32],
                                mybir.dt.bfloat16,
                                isOutput=False)   # conv2 taps (M=32 bands, bf16)
    c1b = din("c1b", (P, 16))              # conv1 bias, per out-chunk columns
    c2b = din("c2b", (P, HC))
    cosT = din("cosT", (P, HC, S))
    sinT = din("sinT", (P, HC, S))
    wq = [din(f"wq{i}", (P, HC, HC, P)) for i in range(L)]
    wk = [din(f"wk{i}", (P, HC, HC, P)) for i in range(L)]
    wv = [din(f"wv{i}", (P, HC, H)) for i in range(L)]
    wo = [din(f"wo{i}", (P, HC, HC, P)) for i in range(L)]
    w1 = din("w1", (P, HC, FC, P))
    b1 = din("b1", (P, FC))
    w2 = din("w2", (P, FC, HC, P))
    ones_c = din("ones_c", (P, 1))
    ones_r = din("ones_r", (1, P))
    # Optional non-trivial params
    opt = {}
    for nm in ("cnn_g", "cnn_b", "ffn_g", "ffn_b"):
        if flags.get(nm):
            opt[nm] = din(nm, (P, HC))
    for i in range(L):
        for nm in (f"attn_g{i}", f"attn_b{i}"):
            if flags.get(nm):
                opt[nm] = din(nm, (P, HC))
        if flags.get(f"ropebq{i}"):
            opt[f"ropebq{i}"] = din(f"ropebq{i}", (P, HC, S))
        if flags.get(f"ropebk{i}"):
            opt[f"ropebk{i}"] = din(f"ropebk{i}", (P, HC, S))
        if flags.get(f"bv{i}"):
            opt[f"bv{i}"] = din(f"bv{i}", (P, NH, HD + 1))
        if flags.get(f"bo{i}"):
            opt[f"bo{i}"] = din(f"bo{i}", (P, HC))
    if flags.get("b2"):
        opt["b2"] = din("b2", (P, HC))

    outT = nc.declare_dram_parameter("outT", [B_LOCAL, P, HC, S], F32,
                                     isOutput=True)

    with tile.TileContext(nc) as tc:
        with (
            tc.tile_pool(name="dram", bufs=1, space="DRAM") as dpool,
            tc.tile_pool(name="gconst", bufs=1) as gpool,
        ):
            cos_sb = gpool.tile([P, HC, S], F32, name="g_cos")
            sin_sb = gpool.tile([P, HC, S], F32, name="g_sin")
            onc_sb = gpool.tile([P, 1], F32R, name="g_onc")
            onr_sb = gpool.tile([1, P], F32R, name="g_onr")
            nc.sync.dma_start(out=cos_sb, in_=cosT[:])
            nc.sync.dma_start(out=sin_sb, in_=sinT[:])
            nc.sync.dma_start(out=onc_sb, in_=ones_c[:].bitcast(F32R))
            nc.sync.dma_start(out=onr_sb, in_=ones_r[:].bitcast(F32R))
            r0_t = [dpool.tile([P, HC, S], F32, name=f"r0_{b}", tag=f"r0_{b}")
                    for b in range(B_LOCAL)]
            h1_t = [dpool.tile([P, HC, S], F32, name=f"h1_{b}", tag=f"h1_{b}")
                    for b in range(B_LOCAL)]
            h2_t = [dpool.tile([P, HC, S], F32, name=f"h2_{b}", tag=f"h2_{b}")
                    for b in range(B_LOCAL)]

            # ---------------- Phase 0: CNN ----------------
            with (
                tc.tile_pool(name="p0w", bufs=1) as wpool,
                tc.tile_pool(name="p0s", bufs=2) as sbuf,
                tc.tile_pool(name="p0p", bufs=1, space="PSUM") as psum,
                tc.tile_pool(name="p0p2", bufs=1, space="PSUM") as psum2,
            ):
                c1w_sb = wpool.tile([P, HC, 5, P], F32R, name="c1w_sb")
                nc.sync.dma_start(out=c1w_sb, in_=c1w[:].bitcast(F32R))
                c2w_sb = wpool.tile([P, HC, HC, 5, 32], mybir.dt.bfloat16,
                                    name="c2w_sb")
                nc.sync.dma_start(out=c2w_sb, in_=c2w[:])
                c1b_sb = wpool.tile([P, 16], F32, name="c1b_sb")
                nc.sync.dma_start(out=c1b_sb, in_=c1b[:])
                c2b_sb = wpool.tile([P, HC], F32, name="c2b_sb")
                nc.sync.dma_start(out=c2b_sb, in_=c2b[:])

                for b in range(B_LOCAL):
                    x_pad = sbuf.tile([P, HC, S + 4], F32R, name="x_pad",
                                      tag="x_pad")
                    nc.vector.memset(x_pad[:, :, 0:2].bitcast(F32), 0.0)
                    nc.vector.memset(x_pad[:, :, S + 2:S + 4].bitcast(F32), 0.0)
                    nc.sync.dma_start(out=x_pad[:, :, 2:S + 2],
                                      in_=xT[b].bitcast(F32R))
                    # conv1: 16 out-chunks (q in [128c,128c+128)), K=32 taps
                    x2_pad = sbuf.tile([P, 16, S + 4], mybir.dt.bfloat16,
                                       name="x2_pad", tag="x2_pad")
                    nc.vector.memset(x2_pad[:, :, 0:2], 0.0)
                    nc.vector.memset(x2_pad[:, :, S + 2:S + 4], 0.0)
                    for cg in range(4):          # group of 4 out-chunks
                        ps_l = []
                        for r in range(4):       # c = 4*cg + r, rotates rows
                            c = 4 * cg + r
                            g = c // 4
                            ps = psum.tile([P, S], F32, name=f"c1p{r}",
                                           tag=f"pp_c1_{r}")
                            ps_l.append(ps)
                        for d in range(5):
                            for r in range(4):
                                c = 4 * cg + r
                                g = c // 4
                                nc.tensor.matmul(
                                    ps_l[r][:, :],
                                    c1w_sb[32 * r:32 * r + 32, g, d, :],
                                    x_pad[32 * r:32 * r + 32, g, d:d + S],
                                    start=(d == 0), stop=(d == 4),
                                    tile_position=(32 * r, 0),
                                )
                        for r in range(4):
                            c = 4 * cg + r
                            nc.scalar.activation(
                                out=x2_pad[:, c, 2:S + 2], in_=ps_l[r][:, :],
                                func=(AF.Identity if gelu_identity
                                      else AF.Gelu),
                                bias=c1b_sb[:, c:c + 1])
                    # conv2: 4 out-chunks, banded K=128 M=32
                    r0_sb = sbuf.tile([P, HC, S], F32, name="r0_sb",
                                      tag="r0_sb")
                    for c in range(HC):
                        ps2_l = [psum2.tile([P, S], F32, name=f"c2p{k}",
                                            tag=f"pp_c2_{k}")
                                 for k in range(4)]
                        for d in range(5):
                            for k in range(4):
                                nc.tensor.matmul(
                                    ps2_l[k][32 * k:32 * k + 32, :],
                                    c2w_sb[:, c, k, d, :],
                                    x2_pad[:, 4 * c + k, d:d + S],
                                    start=(d == 0), stop=(d == 4),
                                    tile_position=(0, 32 * k),
                                )
                        # residual add x (+ conv2 bias if any), per band
                        for k in range(4):
                            sl = slice(32 * k, 32 * k + 32)
                            nc.vector.tensor_add(r0_sb[sl, c, :],
                                                 ps2_l[k][sl, :],
                                                 x_pad[sl, c, 2:S + 2])
                        if flags.get("c2b_nz"):
                            nc.vector.tensor_scalar_add(
                                r0_sb[:, c, :], r0_sb[:, c, :],
                                c2b_sb[:, c:c + 1])
                    nc.sync.dma_start(out=r0_t[b][:], in_=r0_sb)

            # ---------------- Phases 1+2: attention layers ----------------
            for li in range(L):
                with (
                    tc.tile_pool(name=f"a{li}w", bufs=1) as wpool,
                    tc.tile_pool(name=f"a{li}b", bufs=1) as big,
                    tc.tile_pool(name=f"a{li}s", bufs=2) as sbuf,
                    tc.tile_pool(name=f"a{li}e", bufs=2) as epool,
                    tc.tile_pool(name=f"a{li}p", bufs=2, space="PSUM") as psum,
                    tc.tile_pool(name=f"a{li}ps", bufs=2, space="PSUM") as pscore,
                ):
                    wq_sb = wpool.tile([P, HC, HC, P], F32R, name="wq_sb")
                    wk_sb = wpool.tile([P, HC, HC, P], F32R, name="wk_sb")
                    wv_sb = wpool.tile([P, HC, H], F32R, name="wv_sb")
                    wo_sb = wpool.tile([P, HC, HC, P], F32R, name="wo_sb")
                    nc.sync.dma_start(out=wq_sb, in_=wq[li][:].bitcast(F32R))
                    nc.sync.dma_start(out=wk_sb, in_=wk[li][:].bitcast(F32R))
                    nc.sync.dma_start(out=wv_sb, in_=wv[li][:].bitcast(F32R))
                    nc.sync.dma_start(out=wo_sb, in_=wo[li][:].bitcast(F32R))
                    lngb = None
                    if li == 0 and (flags.get("cnn_g") or flags.get("cnn_b")):
                        lngb = _load_gb(nc, wpool, opt, "cnn_g", "cnn_b")
                    lngb2 = None
                    if flags.get(f"attn_g{li}") or flags.get(f"attn_b{li}"):
                        lngb2 = _load_gb(nc, wpool, opt,
                                         f"attn_g{li}", f"attn_b{li}")
                    ropeb = {}
                    for nm in (f"ropebq{li}", f"ropebk{li}"):
                        if flags.get(nm):
                            t = wpool.tile([P, HC, S], F32, name=f"{nm}_sb")
                            nc.sync.dma_start(out=t, in_=opt[nm][:])
                            ropeb[nm] = t
                    bv_sb = None
                    if flags.get(f"bv{li}"):
                        bv_sb = wpool.tile([P, NH, HD + 1], F32,
                                           name=f"bv{li}_sb")
                        nc.sync.dma_start(out=bv_sb, in_=opt[f"bv{li}"][:])
                    bo_sb = None
                    if flags.get(f"bo{li}"):
                        bo_sb = wpool.tile([P, HC], F32, name=f"bo{li}_sb")
                        nc.sync.dma_start(out=bo_sb, in_=opt[f"bo{li}"][:])

                    src_t = r0_t if li == 0 else h1_t
                    dst_t = h1_t if li == 0 else h2_t

                    for b in range(B_LOCAL):
                        r_sb = sbuf.tile([P, HC, S], F32R, name="ar_sb",
                                         tag="ar_sb")
                        nc.sync.dma_start(out=r_sb,
                                          in_=src_t[b][:].bitcast(F32R))
                        if li == 0:
                            h_sb = big.tile([P, HC, S], F32R, name="ah_sb",
                                             tag="ah_sb")
                            _layer_norm_feat(nc, (big, sbuf, psum), r_sb,
                                             onc_sb, onr_sb, h_sb, gb=lngb)
                        else:
                            h_sb = r_sb  # already normalized (h1)

                        # q, qR, k, kR feature-major; rope combine
                        q_rot = sbuf.tile([P, HC, S], F32R, name="q_rot",
                                          tag="q_rot")
                        k_rot = sbuf.tile([P, HC, S], F32R, name="k_rot",
                                          tag="k_rot")
                        for (wa, rot, rbn) in (
                            (wq_sb, q_rot, f"ropebq{li}"),
                            (wk_sb, k_rot, f"ropebk{li}"),
                        ):
                            for oc in range(HC):
                                ps_a = psum.tile([P, S], F32, name="qk_a",
                                                 tag="pp_qkv")
                                for hc in range(HC):
                                    nc.tensor.matmul(
                                        ps_a[:, :], wa[:, hc, oc, :],
                                        h_sb[:, hc, :],
                                        start=(hc == 0), stop=(hc == HC - 1))
                                # pair-swap via DVE stream_shuffle (32-wide
                                # partition permute); rotation sign lives in
                                # the sin table
                                shuf = sbuf.tile([P, S], F32, name="rope_sh",
                                                 tag="rope_sh")
                                nc.vector.stream_shuffle(
                                    shuf, ps_a[:, :],
                                    mask=[i ^ 1 for i in range(32)])
                                t1 = big.tile([P, S], F32, name="rope_t1",
                                              tag="rope_t1")
                                t2 = big.tile([P, S], F32, name="rope_t2",
                                              tag="rope_t2")
                                nc.vector.tensor_mul(t1, ps_a[:, :],
                                                     cos_sb[:, oc, :])
                                nc.gpsimd.tensor_mul(t2, shuf,
                                                     sin_sb[:, oc, :])
                                with nc.allow_low_precision(reason="f32r rot"):
                                    nc.gpsimd.tensor_add(rot[:, oc, :], t1, t2)
                            if rbn in ropeb:
                                with nc.allow_low_precision(reason="f32r rb"):
                                    nc.gpsimd.tensor_add(rot, rot, ropeb[rbn])

                        # v token-major with ones column
                        v_sb = big.tile([P, HC, NH, HD + 1], F32R,
                                         name="v_sb", tag="v_sb")
                        nc.vector.memset(v_sb[:, :, :, HD:HD + 1].bitcast(F32), 1.0)
                        for tc_ in range(HC):
                            ps_v = psum.tile([P, S], F32, name="v_ps",
                                             tag="pp_qkv")
                            for hc in range(HC):
                                nc.tensor.matmul(
                                    ps_v[:, :],
                                    h_sb[:, hc, P * tc_:P * tc_ + P],
                                    wv_sb[:, hc, :],
                                    start=(hc == 0), stop=(hc == HC - 1))
                            with nc.allow_low_precision(reason="f32r v"):
                                nc.scalar.activation(
                                    out=v_sb[:, tc_, :, 0:HD],
                                    in_=ps_v[:, :].rearrange(
                                        "p (h d) -> p h d", d=HD),
                                    func=AF.Copy)
                        if bv_sb is not None:
                            with nc.allow_low_precision(reason="f32r bv"):
                                for tc_ in range(HC):
                                    nc.gpsimd.tensor_add(
                                        v_sb[:, tc_], v_sb[:, tc_], bv_sb)

                        # attention per head
                        ctx_sb = big.tile([P, HC, S], F32R, name="ctx_sb",
                                           tag="ctx_sb")
                        for hh in range(NH):
                            hc2 = hh // 2
                            off = HD * (hh % 2)
                            expT = [epool.tile([P, S], F32R, name=f"expT{kc}",
                                               tag=f"expT{kc}")
                                    for kc in range(HC)]
                            for kc in range(HC):
                                ps_s = pscore.tile([P, S], F32, name="s_ps",
                                                   tag="pp_score")
                                nc.tensor.matmul(
                                    ps_s[:, :],
                                    k_rot[off:off + HD, hc2,
                                          P * kc:P * kc + P],
                                    q_rot[off:off + HD, hc2, :],
                                    start=True, stop=True)
                                with nc.allow_low_precision(reason="f32r exp"):
                                    nc.scalar.activation(
                                        out=expT[kc], in_=ps_s[:, :],
                                        func=AF.Exp)
                            ps_c = pscore.tile([P, S], F32, name="c_ps",
                                               tag="pp_ctx")
                            for kc in range(HC):
                                nc.tensor.matmul(
                                    ps_c[0:HD + 1, :],
                                    v_sb[:, kc, hh, :],
                                    expT[kc][:, :],
                                    start=(kc == 0), stop=(kc == HC - 1))
                            rcp = sbuf.tile([1, S], F32R, name="rcp",
                                            tag="rcp")
                            with nc.allow_low_precision(reason="f32r rcp"):
                                nc.vector.reciprocal(rcp,
                                                     ps_c[HD:HD + 1, :])
                            ps_rb = psum.tile([HD, S], F32, name="rb_ps",
                                              tag="pp_ln")
                            nc.tensor.matmul(ps_rb[:, :], onr_sb[0:1, 0:HD],
                                             rcp[0:1, :], start=True,
                                             stop=True)
                            rcpb = sbuf.tile([HD, S], F32, name="rcpb",
                                             tag="rcpb")
                            nc.scalar.activation(out=rcpb, in_=ps_rb[:, :],
                                                 func=AF.Copy)
                            with nc.allow_low_precision(reason="f32r ctx"):
                                nc.vector.tensor_mul(
                                    ctx_sb[off:off + HD, hc2, :],
                                    ps_c[0:HD, :], rcpb)

                        # out-proj + residual -> r_new; then LN -> h_next
                        rn_sb = big.tile([P, HC, S], F32R, name="rn_sb",
                                          tag="rn_sb")
                        for oc in range(HC):
                            ps_o = psum.tile([P, S], F32, name="o_ps",
                                             tag="pp_qkv")
                            for hc in range(HC):
                                nc.tensor.matmul(
                                    ps_o[:, :], wo_sb[:, hc, oc, :],
                                    ctx_sb[:, hc, :],
                                    start=(hc == 0), stop=(hc == HC - 1))
                            with nc.allow_low_precision(reason="f32r rn"):
                                nc.vector.tensor_add(rn_sb[:, oc, :],
                                                     ps_o[:, :],
                                                     h_sb[:, oc, :])
                            if bo_sb is not None:
                                with nc.allow_low_precision(reason="f32r bo"):
                                    nc.vector.tensor_scalar_add(
                                        rn_sb[:, oc, :], rn_sb[:, oc, :],
                                        bo_sb[:, oc:oc + 1])
                        hn_sb = big.tile([P, HC, S], F32R, name="hn_sb",
                                         tag="hn_sb")
                        _layer_norm_feat(nc, (big, sbuf, psum), rn_sb,
                                         onc_sb, onr_sb, hn_sb, gb=lngb2)
                        nc.sync.dma_start(out=dst_t[b][:].bitcast(F32R),
                                          in_=hn_sb[:, :, :])

            # ---------------- Phase 3: FFN ----------------
            with (
                tc.tile_pool(name="p3w", bufs=1) as wpool,
                tc.tile_pool(name="p3b", bufs=1) as big,
                tc.tile_pool(name="p3s", bufs=2) as sbuf,
                tc.tile_pool(name="p3p", bufs=2, space="PSUM") as psum,
                tc.tile_pool(name="p3pf", bufs=3, space="PSUM") as psumf,
            ):
                w1_sb = wpool.tile([P, HC, FC, P], F32R, name="w1_sb")
                nc.sync.dma_start(out=w1_sb, in_=w1[:].bitcast(F32R))
                w2_sb = wpool.tile([P, FC, HC, P], F32R, name="w2_sb")
                nc.sync.dma_start(out=w2_sb, in_=w2[:].bitcast(F32R))
                b1_sb = wpool.tile([P, FC], F32, name="b1_sb")
                nc.sync.dma_start(out=b1_sb, in_=b1[:])
                b2_sb = None
                if flags.get("b2"):
                    b2_sb = wpool.tile([P, HC], F32, name="b2_sb")
                    nc.sync.dma_start(out=b2_sb, in_=opt["b2"][:])
                lngb3 = None
                if flags.get("ffn_g") or flags.get("ffn_b"):
                    lngb3 = _load_gb(nc, wpool, opt, "ffn_g", "ffn_b")

                for b in range(B_LOCAL):
                    h_sb = sbuf.tile([P, HC, S], F32R, name="fh_sb",
                                     tag="fh_sb")
                    nc.sync.dma_start(out=h_sb, in_=h2_t[b][:].bitcast(F32R))
                    f1_sb = big.tile([P, FC, S], F32R, name="f1_sb",
                                     tag="f1_sb")
                    for oc in range(FC):
                        ps_f = psumf.tile([P, S], F32, name="f1_ps",
                                          tag="pp_f1")
                        for hc in range(HC):
                            nc.tensor.matmul(
                                ps_f[:, :], w1_sb[:, hc, oc, :],
                                h_sb[:, hc, :],
                                start=(hc == 0), stop=(hc == HC - 1))
                        with nc.allow_low_precision(reason="f32r gelu"):
                            nc.scalar.activation(
                                out=f1_sb[:, oc, :], in_=ps_f[:, :],
                                func=(AF.Identity if gelu_identity
                                      else AF.Gelu),
                                bias=b1_sb[:, oc:oc + 1])
                    rn_sb = big.tile([P, HC, S], F32R, name="frn_sb",
                                     tag="frn_sb")
                    for oc in range(HC):
                        ps_2 = psumf.tile([P, S], F32, name="f2_ps",
                                          tag="pp_f2")
                        for hc in range(FC):
                            nc.tensor.matmul(
                                ps_2[:, :], w2_sb[:, hc, oc, :],
                                f1_sb[:, hc, :],
                                start=(hc == 0), stop=(hc == FC - 1))
                        with nc.allow_low_precision(reason="f32r frn"):
                            nc.vector.tensor_add(rn_sb[:, oc, :], ps_2[:, :],
                                                 h_sb[:, oc, :])
                        if b2_sb is not None:
                            with nc.allow_low_precision(reason="f32r b2"):
                                nc.vector.tensor_scalar_add(
                                    rn_sb[:, oc, :], rn_sb[:, oc, :],
                                    b2_sb[:, oc:oc + 1])
                    out_sb = sbuf.tile([P, HC, S], F32, name="out_sb",
                                       tag="out_sb")
                    _layer_norm_feat(nc, (big, sbuf, psum), rn_sb,
                                     onc_sb, onr_sb, out_sb, gb=lngb3)
                    nc.sync.dma_start(out=outT[b], in_=out_sb)

    if split:
        _split_waits(nc)
    return nc


def _load_gb(nc, wpool, opt, gname, bname):
    g_sb = wpool.tile([P, HC], F32, name=f"{gname}_sb")
    b_sb = wpool.tile([P, HC], F32, name=f"{bname}_sb")
    if gname in opt:
        nc.sync.dma_start(out=g_sb, in_=opt[gname][:])
    else:
        nc.vector.memset(g_sb, 1.0)
    if bname in opt:
        nc.sync.dma_start(out=b_sb, in_=opt[bname][:])
    else:
        nc.vector.memset(b_sb, 0.0)
    return (g_sb, b_sb)


# ---------------------------------------------------------------------------
# Host-side preparation
# ---------------------------------------------------------------------------

def _lhsT_tiles(W):
    """W [out, in] -> lhsT tile array [P, in_chunks, out_chunks, P]."""
    o, i = W.shape
    ic, oc = i // P, o // P
    return np.ascontiguousarray(
        W.T.reshape(ic, P, oc, P).transpose(1, 0, 2, 3)).astype(np.float32)


def _feat_major(v):
    """v [H] (or [H]-like per-feature vec) -> [P, HC]."""
    return np.ascontiguousarray(v.reshape(HC, P).T).astype(np.float32)


def _rope_tables():
    inv = 1.0 / (THETA ** (np.arange(0, H, 2, dtype=np.float64) / H))
    t = np.arange(S, dtype=np.float64)
    fr = t[None, :] * inv[:, None]                     # [H/2, S]
    fr2 = np.repeat(fr, 2, axis=0)                     # [H, S]
    cos = np.cos(fr2).astype(np.float32)
    sin = np.sin(fr2).astype(np.float32)

    def fm(tab):                                       # [H, S] -> [P, HC, S]
        return np.ascontiguousarray(
            tab.reshape(HC, P, S).transpose(1, 0, 2)).astype(np.float32)
    return fm(cos), fm(sin)


def _sin_sign():
    sign = np.where((np.arange(H) % 2) == 0, -1.0, 1.0).astype(np.float32)
    return np.ascontiguousarray(
        sign.reshape(HC, P).T)[:, :, None]          # [P, HC, 1]


def _rot_mat():
    R = np.zeros((H, H), dtype=np.float32)
    idx = np.arange(0, H, 2)
    R[idx, idx + 1] = -1.0
    R[idx + 1, idx] = 1.0
    return R


def _conv1_tiles(w):  # w [4H, 1, 5] -> [P, HC, 5, P]
    out = np.zeros((P, HC, 5, P), dtype=np.float32)
    for c in range(16):
        r, g = c % 4, c // 4
        for d in range(5):
            for m in range(P):
                kin = m // 4
                out[32 * r + kin, g, d, m] = w[128 * c + m, 0, d]
    return out


def _conv2_tiles(w):  # w [H, 4, 5] -> [P, HC, HC, 5, 32]
    out = np.zeros((P, HC, HC, 5, 32), dtype=np.float32)
    for c in range(HC):
        for k in range(HC):
            for d in range(5):
                for m in range(32):
                    hh = 128 * c + 32 * k + m
                    for j in range(4):
                        out[4 * m + j, c, k, d, m] = w[hh, j, d]
    return out


def _rope_bias_table(bvec, scale):
    """rope(b)[o, t] table, feature-major [P, HC, S], for nonzero q/k bias."""
    cos, sin = _rope_tables()  # [P, HC, S]
    bf = _feat_major(bvec * scale)          # [P, HC]
    R = _rot_mat()
    rb = (R @ (bvec * scale)).astype(np.float32)
    rbf = _feat_major(rb)
    return bf[:, :, None] * cos + rbf[:, :, None] * sin


_CACHE = {}


def _get_nc(flags_key, flags):
    if flags_key not in _CACHE:
        _CACHE[flags_key] = build_nc(flags)
    return _CACHE[flags_key]


def prepare(inputs):
    """Compute (flags, common input map, xT) from full inputs."""
    inputs = {k: np.asarray(v, dtype=np.float32) for k, v in inputs.items()}
    x = inputs["x"]

    nz = lambda a: bool(np.any(a != 0.0))
    flags = {}
    flags["c2b_nz"] = nz(inputs["conv2_b"])
    if nz(inputs["cnn_ln_g"] - 1.0):
        flags["cnn_g"] = True
    if nz(inputs["cnn_ln_b"]):
        flags["cnn_b"] = True
    if nz(inputs["ffn_ln_g"] - 1.0):
        flags["ffn_g"] = True
    if nz(inputs["ffn_ln_b"]):
        flags["ffn_b"] = True
    if nz(inputs["b2"]):
        flags["b2"] = True
    for i in range(L):
        if nz(inputs["attn_ln_g"][i] - 1.0):
            flags[f"attn_g{i}"] = True
        if nz(inputs["attn_ln_b"][i]):
            flags[f"attn_b{i}"] = True
        if nz(inputs["bq"][i]):
            flags[f"ropebq{i}"] = True
        if nz(inputs["bk"][i]):
            flags[f"ropebk{i}"] = True
        if nz(inputs["bv"][i]):
            flags[f"bv{i}"] = True
        if nz(inputs["bo"][i]):
            flags[f"bo{i}"] = True

    R = _rot_mat()
    cos_fm, sin_fm = _rope_tables()

    common = {
        "c1w": _conv1_tiles(inputs["conv1_w"]),
        "c2w": _conv2_tiles(inputs["conv2_w"]).astype(ml_dtypes.bfloat16),
        "c1b": np.ascontiguousarray(
            inputs["conv1_b"].reshape(16, P).T).astype(np.float32),
        "c2b": _feat_major(inputs["conv2_b"]),
        "cosT": cos_fm,
        # shuffled-pair rotation: sign folded into the sin table
        # (even feature rows get -sin, odd rows +sin)
        "sinT": sin_fm * _sin_sign(),
        "w1": _lhsT_tiles(inputs["W1"]),
        "b1": np.ascontiguousarray(
            inputs["b1"].reshape(FC, P).T).astype(np.float32),
        "w2": _lhsT_tiles(inputs["W2"]),
        "ones_c": np.ones((P, 1), dtype=np.float32),
        "ones_r": np.ones((1, P), dtype=np.float32),
    }
    for i in range(L):
        wq_eff = (inputs["Wq"][i] * SCALE).astype(np.float32)
        common[f"wq{i}"] = _lhsT_tiles(wq_eff)
        common[f"wk{i}"] = _lhsT_tiles(inputs["Wk"][i])
        # wv as rhs: [P, HC, H] with [p, hc, o] = Wv.T[hc*P+p, o]
        common[f"wv{i}"] = np.ascontiguousarray(
            inputs["Wv"][i].T.reshape(HC, P, H).transpose(1, 0, 2)
        ).astype(np.float32)
        common[f"wo{i}"] = _lhsT_tiles(inputs["Wo"][i])
        if flags.get(f"ropebq{i}"):
            common[f"ropebq{i}"] = _rope_bias_table(inputs["bq"][i], SCALE)
        if flags.get(f"ropebk{i}"):
            common[f"ropebk{i}"] = _rope_bias_table(inputs["bk"][i], 1.0)
        if flags.get(f"bv{i}"):
            t = np.zeros((P, NH, HD + 1), dtype=np.float32)
            t[:, :, 0:HD] = np.broadcast_to(
                inputs["bv"][i].reshape(NH, HD)[None, :, :], (P, NH, HD))
            common[f"bv{i}"] = t
        if flags.get(f"bo{i}"):
            common[f"bo{i}"] = _feat_major(inputs["bo"][i])
    if flags.get("b2"):
        common["b2"] = _feat_major(inputs["b2"])
    for nm_src, nm_dst in (("cnn_ln_g", "cnn_g"), ("cnn_ln_b", "cnn_b"),
                           ("ffn_ln_g", "ffn_g"), ("ffn_ln_b", "ffn_b")):
        if flags.get(nm_dst):
            common[nm_dst] = _feat_major(inputs[nm_src])
    for i in range(L):
        if flags.get(f"attn_g{i}"):
            common[f"attn_g{i}"] = _feat_major(inputs["attn_ln_g"][i])
        if flags.get(f"attn_b{i}"):
            common[f"attn_b{i}"] = _feat_major(inputs["attn_ln_b"][i])

    # x [B, S, H] -> feature-major [B, P, HC, S]
    xT = np.ascontiguousarray(
        x.reshape(B, S, HC, P).transpose(0, 3, 2, 1)).astype(np.float32)
    return flags, common, xT


def _run(inputs, trace=False):
    flags, common, xT = prepare(inputs)
    flags_key = tuple(sorted(flags.items()))
    nc = _get_nc(flags_key, flags)

    in_maps = []
    for c in range(N_CORES):
        m = dict(common)
        m["xT"] = np.ascontiguousarray(xT[c * B_LOCAL:(c + 1) * B_LOCAL])
        in_maps.append(m)

    res = run_bass_kernel_spmd(nc, in_maps, list(range(N_CORES)),
                               trace=trace)
    outs = [res.results[c]["outT"] for c in range(N_CORES)]
    o = np.stack(outs, axis=0)            # [C, B_LOCAL, P, HC, S]
    o = o.transpose(0, 1, 4, 3, 2).reshape(B, S, H)
    return np.ascontiguousarray(o), res


def kernel(**inputs):
    return _run(inputs, trace=False)[0]

